# revision 98
# baseline (speedup 1.0000x reference)
"""Trainium2 Bass kernel for batched pairwise squared-euclidean distance
(retrieval_knn): out[b, n, m] = scale/D * sum_d (query[b,n,d] - prototypes[b,m,d])^2
with bs=8, n=4096, m=32, D=128.

Sharding: data-parallel over the batch dim across the 8 NeuronCores (one
batch element per core), per the sharding hint. kernel() takes the FULL
inputs, preps per-core maps on the host, runs the SPMD Bass program via
run_bass_kernel_spmd, and gathers/reassembles the full (8, 4096, 32) output.

Math: out = scale/D * (||q||^2 - 2 q.p + ||p||^2). The dominant cost is
streaming the query; everything is arranged around that:

- The query ships as bf16 (host cast; the norm terms stay exact-fp32-of-bf16,
  so total relative error ~1e-3, dominated by bf16 rounding of q/p).
- The O(M*D) and O(N*D) side terms (pT2 = -2*scale/D*p^T, pns, qns) are host
  input prep; all O(N*M*D) work runs on the TensorEngine.
- The query transpose (needed because the PE contracts over partitions) is
  hybrid (default MM_DTYPE="bf16h", H_PE=4): ONE hardware batched xbar
  DMA-transpose covers tiles H_PE..31 (out[:, t, :] = rows[t*128:(t+1)*128].T,
  HW-verified semantics; the CoreSim model disagrees -- trust hardware),
  while the first H_PE tiles load naturally and PE-transpose via identity
  matmuls in parallel. Two concurrent xbar transposes CORRUPT each other on
  real silicon (shared xbar state; same-ring back-to-back crashes the
  device), so exactly one DmaTransposeAnt instruction exists; plain DMAs
  and PE transposes overlap it safely.
- Per-output-group PSUM is pre-filled with pns via early K=1 matmuls while
  the transpose streams; 32 bf16 matmuls accumulate -2*scale/D*q.p; one fused
  scalar_tensor_tensor epilogue per group adds qn*scale/D; stores go out as
  parallel half-DMAs on different rings in the device-natural [p, t, m]
  layout (host reshape on gather).
- A 1-sync-wait-per-instruction legalizer works around this walrus build's
  "Too many sync wait commands" limit, and a pass removes Tile's conservative
  xbar-vs-DMA pairwise serialization (safe here: no SBUF-to-SBUF DMAs, and
  no DMA touches the transposed tile).

Never feed a DVE op a 0-step broadcast AP that reads PSUM: it takes the
device down (NRT_EXEC_UNIT_UNRECOVERABLE, HW-verified). Broadcast matmul-rhs
APs and SBUF inner-dim broadcasts are fine.
"""

import numpy as np

BS, N, M, D = 8, 4096, 32, 128
P = 128              # partitions
T = N // P           # 32 query tiles of 128
TPG = 4              # tiles per transpose psum-bank group
G = T // TPG         # 8 transpose groups
# query-load chunk sizes (in tiles): small first chunks so compute can start
# as soon as possible (each DMA has ~1.5us completion latency to its sem)
Q_CHUNKS = [8, 8, 8, 4, 4]
SQ_ASSIGN = "pvpvv"   # per-chunk square engine: p=Pool, v=DVE
USE_FOLD2 = True
CHAIN_FOLDS = False   # True: folds run on the same engine as the square
# output groups (in tiles): small last groups so the final out-DMA (and its
# completion latency, which the kernel-exit drain waits on) starts early
O_GROUPS = [16, 16]
MAX_WAITS = 1        # this walrus build allows 1 sync wait per TPB_CTRL inst

# "bf16": the qT psum->SBUF copy casts to bf16 (so the q.p matmul runs in
# bf16), and pT2 is stored bf16. The norm terms stay fp32; only the cross
# term -2*q.p/D picks up ~2e-4 relative error. "fp32" = exact everywhere.
MM_DTYPE = "bf16h"
H_PE = 4             # hybrid: tiles PE-transposed (must be a multiple of 4)

_cache = {}


def _legalize_waits(nc, mybir, max_waits=MAX_WAITS):
    """The walrus build here rejects instructions carrying more than
    MAX_WAITS sync-wait commands. Hoist excess waits onto NOPs inserted
    immediately before the offending instruction on the same engine —
    semantically identical (engine blocks on each wait in program order)."""
    n_fix = 0
    for bb in nc.main_func.blocks:
        new_insts = []
        for inst in bb.instructions:
            si = inst.sync_info
            waits = list(si.on_wait) if si and si.on_wait else []
            if len(waits) > max_waits:
                extra, keep = waits[:-max_waits], waits[-max_waits:]
                si.on_wait = keep
                while extra:
                    chunk, extra = extra[:max_waits], extra[max_waits:]
                    n_fix += 1
                    nop = mybir.InstNoOp(
                        name=f"LW-{inst.name}-{len(new_insts)}",
                        engine=inst.engine,
                        sync_info=mybir.SyncInfo(on_wait=chunk, on_update=[]),
                        text_hint="legalize_waits",
                    )
                    nc.register_instruction(nop, overwrite=True)
                    new_insts.append(nop)
            new_insts.append(inst)
        bb.instructions[:] = new_insts
    return n_fix


def _remove_xbar_serialization(nc, mybir):
    """Tile pairwise-serializes a DMA-transpose against the adjacent DMAs in
    its global DMA order (conservative guard for the xbar ‖ SBUF-to-SBUF-DMA
    hang). This kernel has no SBUF-to-SBUF DMAs, and Tile itself already
    lets non-adjacent DMAs overlap the transpose, so the adjacent-pair waits
    are pure serialization artifacts here:
      - the transpose has no on-chip inputs -> drop its DMAHW waits
      - no DMA touches the transposed tile -> drop plain-DMA waits on the
        transpose's completion lane (keeping lane-recycling waits, which
        coincide with an inc of the same lane)."""
    tp_lanes = set()
    for bb in nc.main_func.blocks:
        for inst in bb.instructions:
            if type(inst).__name__ == "InstDmaTransposeAnt":
                si = inst.sync_info
                if si and si.on_update:
                    tp_lanes.update(u.ant_name for u in si.on_update)
                if si and si.on_wait:
                    si.on_wait = [
                        w for w in si.on_wait
                        if not str(w.ant_name).startswith(("DMAHW", "DMASW"))
                    ]
    if not tp_lanes:
        return
    for bb in nc.main_func.blocks:
        for inst in bb.instructions:
            if type(inst).__name__ != "InstDMACopy":
                continue
            si = inst.sync_info
            if not si or not si.on_wait:
                continue
            incs = {u.ant_name for u in (si.on_update or [])}
            si.on_wait = [
                w for w in si.on_wait
                if not (w.ant_name in tp_lanes and w.ant_name not in incs)
            ]


def build_nc(mm_dtype=MM_DTYPE):
    import concourse.bass as bass
    from concourse import masks, mybir, tile

    f32 = mybir.dt.float32
    mdt = mybir.dt.bfloat16 if mm_dtype == "bf16" else f32

    nc = bass.Bass()
    q_dram = nc.dram_tensor("q", [N, D], f32, kind="ExternalInput")
    p_dram = nc.dram_tensor("p", [M, D], f32, kind="ExternalInput")
    s_dram = nc.dram_tensor("scale", [1], f32, kind="ExternalInput")
    out_dram = nc.dram_tensor("out", [N, M], f32, kind="ExternalOutput")

    # DRAM views: row n = p*T + t  (partition p holds rows p*T .. p*T+T-1,
    # so both q and out are per-partition contiguous in DRAM)
    q_r = q_dram[:].rearrange("(p t) d -> p t d", p=P)
    out_r = out_dram[:].rearrange("(p t) m -> p t m", p=P)

    with tile.TileContext(nc) as tc:
        import contextlib

        with contextlib.ExitStack() as ctx:
            singles = ctx.enter_context(tc.tile_pool(name="singles", bufs=1))
            qpool = ctx.enter_context(tc.tile_pool(name="qpool", bufs=1))
            qtpool = ctx.enter_context(tc.tile_pool(name="qtpool", bufs=3))
            outpool = ctx.enter_context(tc.tile_pool(name="outpool", bufs=1))
            psA = ctx.enter_context(tc.tile_pool(name="psA", bufs=2, space="PSUM"))
            psB = ctx.enter_context(tc.tile_pool(name="psB", bufs=3, space="PSUM"))
            psS = ctx.enter_context(tc.tile_pool(name="psS", bufs=1, space="PSUM"))

            # ---------------- constants / small setup ----------------
            identity = singles.tile([P, P], f32)
            masks.make_identity(nc, identity[:])
            ones_col = singles.tile([P, 1], f32)
            nc.vector.memset(ones_col[:], 1.0)
            ones_row = singles.tile([1, P], f32)
            nc.vector.memset(ones_row[:], 1.0)

            scale_bc = singles.tile([P, 1], f32)
            nc.gpsimd.dma_start(out=scale_bc[:], in_=s_dram[:].to_broadcast([P, 1]))
            sA = singles.tile([P, 1], f32)   # -2*scale/D
            nc.vector.tensor_scalar_mul(sA[:], scale_bc[:], -2.0 / D)
            sB = singles.tile([P, 1], f32)   # scale/D
            nc.vector.tensor_scalar_mul(sB[:], scale_bc[:], 1.0 / D)

            # prototypes: load [M, D], PE-transpose to pT [D, M]
            p_sb = singles.tile([M, D], f32)
            nc.gpsimd.dma_start(out=p_sb[:], in_=p_dram[:])
            psum_pT = psS.tile([P, M], f32, tag="pT")
            nc.tensor.transpose(psum_pT[:], p_sb[:], identity[:M, :M])
            # pT2 = pT * (-2*scale/D), in matmul dtype
            pT2 = singles.tile([P, M], mdt)
            nc.vector.tensor_scalar_mul(pT2[:], psum_pT[:], sA[:])
            # pn_row [1, M] = sum_d pT^2 (unscaled, via ones-column matmul)
            pT_f32 = singles.tile([P, M], f32)
            nc.vector.tensor_copy(pT_f32[:], psum_pT[:])
            pT_sq = singles.tile([P, M], f32)
            nc.vector.tensor_mul(pT_sq[:], pT_f32[:], pT_f32[:])
            psum_pn = psS.tile([1, M], f32, tag="pn")
            nc.tensor.matmul(psum_pn[:], ones_col[:], pT_sq[:], start=True, stop=True)
            pn_row = singles.tile([1, M], f32)
            nc.vector.tensor_copy(pn_row[:], psum_pn[:])
            # pnb [P, M] = broadcast of pn over partitions (K=1 matmul), * scale/D
            psum_pnb = psS.tile([P, M], f32, tag="pnb")
            nc.tensor.matmul(psum_pnb[:], ones_row[:], pn_row[:], start=True, stop=True)
            pnb = singles.tile([P, M], f32)
            nc.vector.tensor_scalar_mul(pnb[:], psum_pnb[:], sB[:])

            # ---------------- query load + qnorm + main loop, interleaved ----
            # Emission order = Tile scheduler priority = per-engine FIFO order,
            # so emit each piece of work as soon as its data chunk can be in
            # flight: chunk DMA -> qnorm chain -> transpose/copy/matmul groups
            # covered by data so far -> epilogue + store per output group.
            q_sb = qpool.tile([P, T, D], f32)
            qn_s = singles.tile([P, T], f32)      # ||q||^2 per (p, t)
            q_sq = qpool.tile([P, T, D], f32)
            q_fold = qpool.tile([P, T, D // 2], f32)
            combined = singles.tile([P, T, M], f32)  # scale/D*qn + scale/D*pn
            q_fold2 = qpool.tile([P, T, D // 4], f32)
            out_sb = outpool.tile([P, T, M], f32)

            o_bounds = []
            acc = 0
            for osz in O_GROUPS:
                o_bounds.append((acc, acc + osz))
                acc += osz
            assert acc == T

            psum_o_tiles = {}
            tiles_ready = 0   # tiles whose chunk DMA has been emitted
            tiles_mm = 0      # tiles whose matmul has been emitted
            go_done = 0

            def emit_chunk(c, csz, c0):
                sl = slice(c0, c0 + csz)
                dma_eng = nc.sync if c % 2 == 0 else nc.scalar
                dma_eng.dma_start(out=q_sb[:, sl, :], in_=q_r[:, sl, :])
                # square + two half-folds on Pool (otherwise idle), reduce on DVE
                nc.gpsimd.tensor_mul(q_sq[:, sl, :], q_sb[:, sl, :], q_sb[:, sl, :])
                nc.gpsimd.tensor_add(
                    q_fold[:, sl, :], q_sq[:, sl, 0:D // 2], q_sq[:, sl, D // 2:D]
                )
                nc.gpsimd.tensor_add(
                    q_fold2[:, sl, :],
                    q_fold[:, sl, 0:D // 4], q_fold[:, sl, D // 4:D // 2],
                )
                nc.vector.tensor_reduce(
                    qn_s[:, sl], q_fold2[:, sl, :],
                    axis=mybir.AxisListType.X, op=mybir.AluOpType.add,
                )
                # combined[p, t, m] = qn_s[p, t]*scale/D + pnb[p, m]
                nc.vector.scalar_tensor_tensor(
                    out=combined[:, sl, :],
                    in0=qn_s[:, sl].unsqueeze(2).to_broadcast([P, csz, M]),
                    scalar=sB[:],
                    in1=pnb[:].unsqueeze(1).to_broadcast([P, csz, M]),
                    op0=mybir.AluOpType.mult,
                    op1=mybir.AluOpType.add,
                )

            def emit_group(t, gsz):
                # transposes -> one ACT copy (casts to mm dtype) -> matmuls
                psum_qT = psA.tile([P, gsz * P], bf16, tag="qT")
                for j in range(gsz):
                    nc.tensor.transpose(
                        psum_qT[:, j * P:(j + 1) * P], q_sb[:, t + j, :], identity[:]
                    )
                qT_sb = qtpool.tile([P, gsz * P], mdt, tag="qT_sb")
                nc.scalar.copy(qT_sb[:], psum_qT[:])
                for j in range(gsz):
                    go = next(
                        i for i, (a, b) in enumerate(o_bounds) if a <= t + j < b
                    )
                    if go not in psum_o_tiles:
                        psum_o_tiles[go] = psB.tile(
                            [P, O_GROUPS[go] * M], f32, tag="o",
                            name=f"psum_o_{go}",
                        )
                    a, _ = o_bounds[go]
                    nc.tensor.matmul(
                        psum_o_tiles[go][:, (t + j - a) * M:(t + j - a + 1) * M],
                        qT_sb[:, j * P:(j + 1) * P],
                        pT2[:],
                        start=True, stop=True,
                    )

            def emit_store(go):
                a, b = o_bounds[go]
                osl = slice(a, b)
                nc.vector.tensor_tensor(
                    out=out_sb[:, osl, :],
                    in0=psum_o_tiles[go][:].rearrange("p (t m) -> p t m", m=M),
                    in1=combined[:, osl, :],
                    op=mybir.AluOpType.add,
                )
                # the very last store goes on the other ring so the two final
                # DMAs (and their completion latencies) overlap
                eng = nc.scalar if go == len(O_GROUPS) - 1 else nc.sync
                eng.dma_start(out=out_r[:, osl, :], in_=out_sb[:, osl, :])

            c_start = 0
            for c, csz in enumerate(Q_CHUNKS):
                emit_chunk(c, csz, c_start)
                c_start += csz
                tiles_ready += csz
                # emit every full transpose group now covered by loaded chunks
                while tiles_mm + TPG <= tiles_ready:
                    emit_group(tiles_mm, TPG)
                    tiles_mm += TPG
                    while (
                        go_done < len(O_GROUPS)
                        and o_bounds[go_done][1] <= tiles_mm
                    ):
                        emit_store(go_done)
                        go_done += 1
            assert tiles_mm == T and go_done == len(O_GROUPS)

    _legalize_waits(nc, mybir)
    return nc


def build_nc_bf16x():
    """v8: host passes q as bf16, pT2 = -2*p.T (bf16), pns = ||p||^2, and
    qns[n, t] = ||q_bf[t*128+n]||^2 pre-transposed (f32) -- the qnorm of the
    bf16-rounded query values, so device results match exactly.
    Device work: ONE hardware batched DMA-transpose of the whole query
    (qT_all[:, t, :] = q[t*128:(t+1)*128].T, HW-verified semantics),
    per-group PSUM pre-filled with pns via K=1 matmuls, 32 bf16 matmuls
    accumulating -2*q.p, and a fused epilogue adding qn and applying the
    runtime scale/D. Everything else is tiny setup DMA."""
    import concourse.bass as bass
    from concourse import mybir, tile

    f32 = mybir.dt.float32
    bf16 = mybir.dt.bfloat16

    nc = bass.Bass()
    q_dram = nc.dram_tensor("q", [N, D], bf16, kind="ExternalInput")
    pt2_dram = nc.dram_tensor("pT2", [D, M], bf16, kind="ExternalInput")
    pns_dram = nc.dram_tensor("pns", [M], f32, kind="ExternalInput")
    qns_dram = nc.dram_tensor("qns", [P, T], f32, kind="ExternalInput")
    s_dram = nc.dram_tensor("scale", [1], f32, kind="ExternalInput")
    # device-natural output layout [p, t, m] (per-partition contiguous DMA);
    # host unshuffles to [t*128+p, m] during the gather
    out_dram = nc.dram_tensor("out", [P, T, M], f32, kind="ExternalOutput")
    out_r = out_dram[:]

    with tile.TileContext(nc) as tc:
        import contextlib

        with contextlib.ExitStack() as ctx:
            singles = ctx.enter_context(tc.tile_pool(name="singles", bufs=1))
            qpool = ctx.enter_context(tc.tile_pool(name="qpool", bufs=1))
            outpool = ctx.enter_context(tc.tile_pool(name="outpool", bufs=1))
            psB = ctx.enter_context(tc.tile_pool(name="psB", bufs=1, space="PSUM"))

            qT_all = qpool.tile([P, T, D], bf16)     # [d, t, nq]
            qn_s = singles.tile([P, T], f32)
            out_sb = outpool.tile([P, T, M], f32)

            # tiny loads first (their consumers sit on the critical path),
            # then the transpose halves on both HWDGE rings (HW batched
            # semantics: tile t = contiguous rows, so each half delivers 16
            # complete tiles)
            ones_row = singles.tile([1, P], f32)
            nc.vector.memset(ones_row[:], 1.0)
            # ONE transpose instruction: concurrent DMA-transposes corrupt
            # each other on real hardware (shared xbar state across the SDMA
            # engines; HW-verified), so the whole query goes in a single
            # xbar stream, alone on the sync ring. Plain DMAs overlapping it
            # are safe (HW-verified).
            pn_sb = singles.tile([1, M], f32)
            nc.scalar.dma_start(out=pn_sb[:], in_=pns_dram[:].unsqueeze(0))
            nc.scalar.dma_start(out=qn_s[:], in_=qns_dram[:])
            nc.sync.dma_start_transpose(qT_all[:], q_dram[:])
            scale_bc = singles.tile([P, 1], f32)
            nc.gpsimd.dma_start(out=scale_bc[:], in_=s_dram[:].to_broadcast([P, 1]))
            sB = singles.tile([P, 1], f32)   # scale/D
            nc.vector.tensor_scalar_mul(sB[:], scale_bc[:], 1.0 / D)
            pT2 = singles.tile([P, M], bf16)
            nc.gpsimd.dma_start(out=pT2[:], in_=pt2_dram[:])


            o_bounds = []
            acc = 0
            for osz in O_GROUPS:
                o_bounds.append((acc, acc + osz))
                acc += osz
            assert acc == T

            # per-group psum pre-filled with pns via cheap bf16 K=1 matmuls
            # (broadcast matmul-rhs APs are HW-proven); tile matmuls then
            # accumulate -2*scale/D*q.p on top
            psum_o_tiles = {}
            for go, (a, b) in enumerate(o_bounds):
                psum_o_tiles[go] = psB.tile(
                    [P, b - a, M], f32, tag=f"o{go}", name=f"psum_o_{go}"
                )
                nc.tensor.matmul(
                    psum_o_tiles[go][:],
                    ones_row[:],
                    pn_sb[:].unsqueeze(1).to_broadcast([1, b - a, M]),
                    start=True, stop=False,
                    skip_group_check=True,
                )

            def emit_mms(go):
                a, b = o_bounds[go]
                for t in range(a, b):
                    nc.tensor.matmul(
                        psum_o_tiles[go][:, t - a, :],
                        qT_all[:, t, :],
                        pT2[:],
                        start=False, stop=(t == b - 1),
                        skip_group_check=True,
                    )

            def emit_store(go):
                a, b = o_bounds[go]
                osl = slice(a, b)
                # out = qn*scale/D + (pns - 2*scale/D*q.p)
                nc.vector.scalar_tensor_tensor(
                    out=out_sb[:, osl, :],
                    in0=qn_s[:, osl].unsqueeze(2).to_broadcast([P, b - a, M]),
                    scalar=sB[:],
                    in1=psum_o_tiles[go][:],
                    op0=mybir.AluOpType.mult,
                    op1=mybir.AluOpType.add,
                )
                # two parallel half-stores on different rings: halves the
                # final DMA slice on the critical tail
                h = (a + b) // 2
                ring_pairs = [(nc.scalar, nc.gpsimd), (nc.sync, nc.scalar)]
                e1, e2 = ring_pairs[go % len(ring_pairs)]
                e1.dma_start(out=out_r[:, a:h, :], in_=out_sb[:, a:h, :])
                e2.dma_start(out=out_r[:, h:b, :], in_=out_sb[:, h:b, :])

            for go in range(len(O_GROUPS)):
                emit_mms(go)
                emit_store(go)

    _remove_xbar_serialization(nc, mybir)
    _legalize_waits(nc, mybir)
    return nc


def build_nc_bf16p():
    """v11: PE-transpose variant. Query loads contiguously (partition p =
    rows p*32+t, cheap 8KB/partition descriptors, ~0.6us DMA receipts),
    PE transposes 4-tile groups via identity matmuls into PSUM, one
    psum->SBUF bf16 copy per group (DVE/ACT alternating), then bf16
    matmuls accumulate onto pns-pre-filled PSUM. Host supplies qns/pns/pT2
    like bf16x but with the mod-32 row mapping (qns = qn.reshape(P, T));
    the output unshuffle is a plain reshape."""
    import concourse.bass as bass
    from concourse import masks, mybir, tile

    f32 = mybir.dt.float32
    bf16 = mybir.dt.bfloat16
    TPG_ = 4

    nc = bass.Bass()
    q_dram = nc.dram_tensor("q", [N, D], bf16, kind="ExternalInput")
    pt2_dram = nc.dram_tensor("pT2", [D, M], bf16, kind="ExternalInput")
    pns_dram = nc.dram_tensor("pns", [M], f32, kind="ExternalInput")
    qns_dram = nc.dram_tensor("qns", [P, T], f32, kind="ExternalInput")
    s_dram = nc.dram_tensor("scale", [1], f32, kind="ExternalInput")
    # device-natural out [p, t, m]; row p*32+t, so host just reshapes
    out_dram = nc.dram_tensor("out", [P, T, M], f32, kind="ExternalOutput")

    # partition p holds rows p*T .. p*T+T-1 (contiguous per partition)
    q_r = q_dram[:].rearrange("(p t) d -> p t d", p=P)
    out_r = out_dram[:]

    with tile.TileContext(nc) as tc:
        import contextlib

        with contextlib.ExitStack() as ctx:
            singles = ctx.enter_context(tc.tile_pool(name="singles", bufs=1))
            qpool = ctx.enter_context(tc.tile_pool(name="qpool", bufs=1))
            outpool = ctx.enter_context(tc.tile_pool(name="outpool", bufs=1))
            qtpool = ctx.enter_context(tc.tile_pool(name="qtpool", bufs=3))
            psA = ctx.enter_context(tc.tile_pool(name="psA", bufs=3, space="PSUM"))
            psB = ctx.enter_context(tc.tile_pool(name="psB", bufs=1, space="PSUM"))

            q_bf = qpool.tile([P, T, D], bf16)
            qn_s = singles.tile([P, T], f32)
            out_sb = outpool.tile([P, T, M], f32)

            # tiny loads + setup
            ones_row = singles.tile([1, P], f32)
            nc.vector.memset(ones_row[:], 1.0)
            # warm the ACT function table right away (otherwise the first
            # psum->sbuf copy on ACT pays the ~1.4us table load mid-pipeline)
            act_warm = singles.tile([1, P], f32)
            nc.scalar.copy(act_warm[:], ones_row[:])
            identity = singles.tile([P, P], bf16)
            masks.make_identity(nc, identity[:])
            pn_sb = singles.tile([1, M], f32)
            nc.gpsimd.dma_start(out=pn_sb[:], in_=pns_dram[:].unsqueeze(0))
            scale_bc = singles.tile([P, 1], f32)
            nc.gpsimd.dma_start(out=scale_bc[:], in_=s_dram[:].to_broadcast([P, 1]))
            sB = singles.tile([P, 1], f32)   # scale/D
            nc.vector.tensor_scalar_mul(sB[:], scale_bc[:], 1.0 / D)
            pT2 = singles.tile([P, M], bf16)
            nc.gpsimd.dma_start(out=pT2[:], in_=pt2_dram[:])
            nc.scalar.dma_start(out=qn_s[:], in_=qns_dram[:])

            o_bounds = []
            acc = 0
            for osz in O_GROUPS:
                o_bounds.append((acc, acc + osz))
                acc += osz
            assert acc == T

            psum_o_tiles = {}
            for go, (a, b) in enumerate(o_bounds):
                psum_o_tiles[go] = psB.tile(
                    [P, b - a, M], f32, tag=f"o{go}", name=f"psum_o_{go}"
                )
                nc.tensor.matmul(
                    psum_o_tiles[go][:],
                    ones_row[:],
                    pn_sb[:].unsqueeze(1).to_broadcast([1, b - a, M]),
                    start=True, stop=False,
                    skip_group_check=True,
                )

            def which_group(t):
                return next(
                    i for i, (a, b) in enumerate(o_bounds) if a <= t < b
                )

            copy_idx = 0

            def emit_tile_group(t0, gsz):
                nonlocal copy_idx
                psum_qT = psA.tile([P, gsz * P], bf16, tag="qT")
                for j in range(gsz):
                    nc.tensor.transpose(
                        psum_qT[:, j * P:(j + 1) * P],
                        q_bf[:, t0 + j, :], identity[:],
                    )
                qT_sb = qtpool.tile([P, gsz * P], bf16, tag="qT_sb")
                if copy_idx % 2 == 0:
                    nc.vector.tensor_copy(qT_sb[:], psum_qT[:])
                else:
                    nc.scalar.copy(qT_sb[:], psum_qT[:])
                copy_idx += 1
                for j in range(gsz):
                    t = t0 + j
                    go = which_group(t)
                    a, b = o_bounds[go]
                    nc.tensor.matmul(
                        psum_o_tiles[go][:, t - a, :],
                        qT_sb[:, j * P:(j + 1) * P],
                        pT2[:],
                        start=False, stop=(t == b - 1),
                        skip_group_check=True,
                    )

            def emit_store(go):
                a, b = o_bounds[go]
                osl = slice(a, b)
                nc.vector.scalar_tensor_tensor(
                    out=out_sb[:, osl, :],
                    in0=qn_s[:, osl].unsqueeze(2).to_broadcast([P, b - a, M]),
                    scalar=sB[:],
                    in1=psum_o_tiles[go][:],
                    op0=mybir.AluOpType.mult,
                    op1=mybir.AluOpType.add,
                )
                h = (a + b) // 2
                ring_pairs = [(nc.scalar, nc.gpsimd), (nc.sync, nc.scalar)]
                e1, e2 = ring_pairs[go % len(ring_pairs)]
                e1.dma_start(out=out_r[:, a:h, :], in_=out_sb[:, a:h, :])
                e2.dma_start(out=out_r[:, h:b, :], in_=out_sb[:, h:b, :])

            # chunk loads split across both HWDGE rings, tile groups emitted
            # as soon as their chunk is in flight
            c_start = 0
            t_done = 0
            go_done = 0
            for c, csz in enumerate(Q_CHUNKS):
                sl = slice(c_start, c_start + csz)
                eng = nc.sync if c % 2 == 0 else nc.scalar
                eng.dma_start(out=q_bf[:, sl, :], in_=q_r[:, sl, :])
                c_start += csz
                while t_done + TPG_ <= c_start:
                    emit_tile_group(t_done, TPG_)
                    t_done += TPG_
                while (
                    go_done < len(O_GROUPS)
                    and o_bounds[go_done][1] <= t_done
                ):
                    emit_store(go_done)
                    go_done += 1
            assert t_done == T and go_done == len(O_GROUPS)

    _legalize_waits(nc, mybir)
    return nc


def build_nc_bf16h():
    """v12 hybrid: ONE xbar DMA-transpose (tiles 16-31 = query rows
    [2048:4096), batched HW semantics) runs concurrently with PE transposes
    of tiles 0-15, which load naturally/contiguously (partition p = rows
    p*16+t', 4KB/partition). Halving the xbar stream halves its completion
    time, and the PE path finishes in parallel. Only one DmaTransposeAnt
    exists (concurrent xbar streams corrupt on HW); everything else is plain
    DMA + HW-proven compute ops."""
    import concourse.bass as bass
    from concourse import masks, mybir, tile

    f32 = mybir.dt.float32
    bf16 = mybir.dt.bfloat16
    HP = H_PE           # PE-transposed tiles (rows [0 : HP*128))
    HX = T - HP         # xbar tiles
    NH = HP * P

    nc = bass.Bass()
    q_dram = nc.dram_tensor("q", [N, D], bf16, kind="ExternalInput")
    pt2_dram = nc.dram_tensor("pT2", [D, M], bf16, kind="ExternalInput")
    pns_dram = nc.dram_tensor("pns", [M], bf16, kind="ExternalInput")
    qns_dram = nc.dram_tensor("qns", [P, T], f32, kind="ExternalInput")
    s_dram = nc.dram_tensor("scale", [1], f32, kind="ExternalInput")
    # out[p, t, m]: tiles 0-15 hold row p*16+t (PE half, rows [0:2048)),
    # tiles 16-31 hold row 2048 + (t-16)*128 + p (xbar half)
    out_dram = nc.dram_tensor("out", [P, T, M], f32, kind="ExternalOutput")
    out_r = out_dram[:]

    # natural view of the PE half: partition p = rows p*HP .. p*HP+HP-1
    q_nat = q_dram[0:NH, :].rearrange("(p t) d -> p t d", p=P)

    with tile.TileContext(nc) as tc:
        import contextlib

        with contextlib.ExitStack() as ctx:
            singles = ctx.enter_context(tc.tile_pool(name="singles", bufs=1))
            qpool = ctx.enter_context(tc.tile_pool(name="qpool", bufs=1))
            outpool = ctx.enter_context(tc.tile_pool(name="outpool", bufs=1))
            qtpool = ctx.enter_context(tc.tile_pool(name="qtpool", bufs=3))
            psA = ctx.enter_context(tc.tile_pool(name="psA", bufs=3, space="PSUM"))
            psB = ctx.enter_context(tc.tile_pool(name="psB", bufs=1, space="PSUM"))

            q_bf = qpool.tile([P, HP, D], bf16)      # natural, PE half
            qT_x = qpool.tile([P, HX, D], bf16)      # xbar half [d, tx, n]
            qn_s = singles.tile([P, T], f32)
            out_sb = outpool.tile([P, T, M], f32)

            # pns first on the scalar ring (feeds the early PSUM fills),
            # then the natural chunks; the single xbar transpose alone on
            # the sync ring
            pn_sb = singles.tile([1, M], bf16)
            nc.scalar.dma_start(out=pn_sb[:], in_=pns_dram[:].unsqueeze(0))
            nc.scalar.dma_start(out=q_bf[:, 0:HP // 2, :], in_=q_nat[:, 0:HP // 2, :])
            nc.scalar.dma_start(out=q_bf[:, HP // 2:HP, :], in_=q_nat[:, HP // 2:HP, :])
            nc.sync.dma_start_transpose(qT_x[:], q_dram[NH:N, :])

            ones_row = singles.tile([1, P], bf16)
            nc.vector.memset(ones_row[:], 1.0)
            act_warm = singles.tile([1, P], f32)
            nc.scalar.copy(act_warm[:], ones_row[:])
            identity = singles.tile([P, P], bf16)
            masks.make_identity(nc, identity[:])
            scale_bc = singles.tile([P, 1], f32)
            nc.gpsimd.dma_start(out=scale_bc[:], in_=s_dram[:].to_broadcast([P, 1]))
            sB = singles.tile([P, 1], f32)   # scale/D
            nc.vector.tensor_scalar_mul(sB[:], scale_bc[:], 1.0 / D)
            pT2 = singles.tile([P, M], bf16)
            nc.gpsimd.dma_start(out=pT2[:], in_=pt2_dram[:])
            nc.gpsimd.dma_start(out=qn_s[:], in_=qns_dram[:])

            # two output groups = the two halves; psum pre-filled with pns
            sizes = {0: HP, 1: HX}
            psum_o = {}
            for go in range(2):
                psum_o[go] = psB.tile(
                    [P, sizes[go], M], f32, tag=f"o{go}", name=f"psum_o_{go}"
                )
                # fills may not cross a PSUM bank (N<=512 fp32): split at 16
                f0 = 0
                while f0 < sizes[go]:
                    fsz = min(16, sizes[go] - f0)
                    nc.tensor.matmul(
                        psum_o[go][:, f0:f0 + fsz, :],
                        ones_row[:],
                        pn_sb[:].unsqueeze(1).to_broadcast([1, fsz, M]),
                        start=True, stop=False,
                        skip_group_check=True,
                    )
                    f0 += fsz

            # PE half: per 4-tile group, transposes -> one bf16 copy -> mms
            for g in range(HP // 4):
                psum_qT = psA.tile([P, 4 * P], bf16, tag="qT")
                for j in range(4):
                    nc.tensor.transpose(
                        psum_qT[:, j * P:(j + 1) * P],
                        q_bf[:, g * 4 + j, :], identity[:],
                    )
                qT_sb = qtpool.tile([P, 4 * P], bf16, tag="qT_sb")
                if g % 2 == 0:
                    nc.vector.tensor_copy(qT_sb[:], psum_qT[:])
                else:
                    nc.scalar.copy(qT_sb[:], psum_qT[:])
                for j in range(4):
                    t = g * 4 + j
                    nc.tensor.matmul(
                        psum_o[0][:, t, :],
                        qT_sb[:, j * P:(j + 1) * P],
                        pT2[:],
                        start=False, stop=(t == HP - 1),
                        skip_group_check=True,
                    )

            # xbar half: mms straight off the transposed tile
            for tx in range(HX):
                nc.tensor.matmul(
                    psum_o[1][:, tx, :],
                    qT_x[:, tx, :],
                    pT2[:],
                    start=False, stop=(tx == HX - 1),
                    skip_group_check=True,
                )

            def emit_store(go, lo, hi):
                a0 = 0 if go == 0 else HP
                a, b = a0 + lo, a0 + hi
                osl = slice(a, b)
                nc.vector.scalar_tensor_tensor(
                    out=out_sb[:, osl, :],
                    in0=qn_s[:, osl].unsqueeze(2).to_broadcast([P, b - a, M]),
                    scalar=sB[:],
                    in1=psum_o[go][:, lo:hi, :],
                    op0=mybir.AluOpType.mult,
                    op1=mybir.AluOpType.add,
                )
                if b - a > 16:
                    # big (xbar) half: 3 parallel stores on 3 rings
                    c1 = a + (b - a) // 3
                    c2 = a + 2 * (b - a) // 3
                    nc.scalar.dma_start(out=out_r[:, a:c1, :], in_=out_sb[:, a:c1, :])
                    nc.sync.dma_start(out=out_r[:, c1:c2, :], in_=out_sb[:, c1:c2, :])
                    nc.gpsimd.dma_start(out=out_r[:, c2:b, :], in_=out_sb[:, c2:b, :])
                else:
                    h = (a + b) // 2
                    ring_pairs = [(nc.scalar, nc.gpsimd), (nc.sync, nc.scalar)]
                    e1, e2 = ring_pairs[emit_store.k % 2]
                    emit_store.k += 1
                    e1.dma_start(out=out_r[:, a:h, :], in_=out_sb[:, a:h, :])
                    e2.dma_start(out=out_r[:, h:b, :], in_=out_sb[:, h:b, :])

            emit_store.k = 0
            emit_store(0, 0, HP)
            emit_store(1, 0, HX)

            # (the xbar-half store is re-split 3 ways inside emit_store)

    _remove_xbar_serialization(nc, mybir)
    _legalize_waits(nc, mybir)
    return nc


def prep_inputs_bf16h(query, prototypes, scale):
    """Host prep for the hybrid: tiles 0-15 use row p*16+t (PE half over
    rows [0:2048)), tiles 16-31 use row 2048+(t-16)*128+n (xbar half)."""
    import ml_dtypes

    maps = prep_inputs_bf16x(query, prototypes, scale)
    HP, HX = H_PE, T - H_PE
    NH = HP * P
    for m in maps:
        m["pns"] = m["pns"].astype(ml_dtypes.bfloat16)
        qn = (m["q"].astype(np.float32) ** 2).sum(-1)   # [N]
        first = qn[:NH].reshape(P, HP)                   # [p, t']
        second = qn[NH:].reshape(HX, P).T                # [n, tx]
        m["qns"] = np.ascontiguousarray(
            np.concatenate([first, second], axis=1)
        ).astype(np.float32)
    return maps


def prep_inputs_bf16x(query, prototypes, scale):
    """Host-side input prep for the bf16x kernel (per-batch maps)."""
    import ml_dtypes

    query = np.asarray(query, dtype=np.float32)
    prototypes = np.asarray(prototypes, dtype=np.float32)
    s = float(np.asarray(scale, dtype=np.float32).reshape(()))
    q_bf = np.ascontiguousarray(query.astype(ml_dtypes.bfloat16))
    pt2 = np.ascontiguousarray(
        (-2.0 * s / D) * prototypes.transpose(0, 2, 1)
    ).astype(ml_dtypes.bfloat16)
    pns = np.ascontiguousarray(
        (s / D) * (prototypes.astype(np.float64) ** 2).sum(-1)
    ).astype(np.float32)
    # qns[b, n, t] = ||q_bf[b, t*128+n]||^2 over the bf16-rounded values
    qf = q_bf.astype(np.float32)
    qn = (qf * qf).sum(-1)                       # [BS, N]
    qns = np.ascontiguousarray(
        qn.reshape(BS, T, P).transpose(0, 2, 1)  # [BS, P(n), T(t)]
    ).astype(np.float32)
    scale_np = np.asarray([s], dtype=np.float32)
    return [
        {"q": q_bf[bb], "pT2": pt2[bb], "pns": pns[bb], "qns": qns[bb],
         "scale": scale_np}
        for bb in range(BS)
    ]


def prep_inputs_bf16p(query, prototypes, scale):
    """Host prep for the PE-transpose variant: row mapping r = p*32 + t."""
    import ml_dtypes

    maps = prep_inputs_bf16x(query, prototypes, scale)
    for m in maps:
        qn = (m["q"].astype(np.float32) ** 2).sum(-1)   # [N], row-major
        m["qns"] = np.ascontiguousarray(qn.reshape(P, T)).astype(np.float32)
    return maps


def kernel(prototypes, masktypes, query, support, support_labels, n_way, n_shot,
           scale, **_ignored):
    from concourse.bass_utils import run_bass_kernel_spmd

    key = ("nc", MM_DTYPE)
    if key not in _cache:
        _cache[key] = (
            build_nc_bf16h() if MM_DTYPE == "bf16h"
            else build_nc_bf16p() if MM_DTYPE == "bf16p"
            else build_nc_bf16x() if MM_DTYPE == "bf16x"
            else build_nc(MM_DTYPE)
        )
    nc = _cache[key]

    if MM_DTYPE == "bf16h":
        in_maps = prep_inputs_bf16h(query, prototypes, scale)
    elif MM_DTYPE == "bf16p":
        in_maps = prep_inputs_bf16p(query, prototypes, scale)
    elif MM_DTYPE == "bf16x":
        in_maps = prep_inputs_bf16x(query, prototypes, scale)
    else:
        query = np.ascontiguousarray(np.asarray(query, dtype=np.float32))
        prototypes = np.ascontiguousarray(
            np.asarray(prototypes, dtype=np.float32)
        )
        scale_np = np.ascontiguousarray(
            np.asarray(scale, dtype=np.float32).reshape(1)
        )
        in_maps = [
            {"q": query[b], "p": prototypes[b], "scale": scale_np}
            for b in range(BS)
        ]
    res = run_bass_kernel_spmd(nc, in_maps, core_ids=list(range(BS)))
    outs = []
    for b in range(BS):
        o = res.results[b]["out"]
        if MM_DTYPE == "bf16x":
            # [p, t, m] -> row t*128+p
            o = o.reshape(P, T, M).transpose(1, 0, 2).reshape(N, M)
        elif MM_DTYPE == "bf16p":
            # [p, t, m] -> row p*32+t: plain reshape
            o = o.reshape(N, M)
        elif MM_DTYPE == "bf16h":
            o = o.reshape(P, T, M)
            first = o[:, :H_PE, :].reshape(H_PE * P, M)
            second = o[:, H_PE:, :].transpose(1, 0, 2).reshape(
                (T - H_PE) * P, M
            )
            o = np.concatenate([first, second], axis=0)
        outs.append(o)
    return np.stack(outs, axis=0).astype(np.float32)
